# revision 12
# baseline (speedup 1.0000x reference)
"""Trainium2 Bass kernel for nn_MemoryUnit (scatter_memory).

Computes: att = softmax(x @ mem.T / 0.5); att = hard_shrink_relu(att, 0.005);
att = att / max(L1(att), eps); out = att @ mem.

Sharding: data-parallel over N across 8 cores; mem replicated per core.

Per 128-row tile (m = 2048 memory slots), with the softmax denominator and
any per-row positive scale cancelled by the L1 renormalization:
  logits = x16 @ mem16.T            (single fp16 matmul, fp32 PSUM accum)
  e = exp(2*logits)                 (f16; ACT, rowsums via ACT accumulators)
  t = lam * rowsum(e)
  g = e * (e > t)                   (one DVE pass, accum S = rowsum(g))
  out = (g^T)^T @ mem / max(S, tiny)

Engine placement:
  - PE: mm1 + mm2 only (mm2 uses gT chunks as the stationary operand so the
    output is out[n, z] directly - no output transpose).
  - DMA XBAR (sync queue): x pre-transpose in 16-tile chunks, g transpose.
  - ACT: the two exp halves + their accumulator reads (rowsums).
  - DVE: the mask pass (scalar_tensor_tensor), reciprocal, final scale.
  - GPSIMD: f32->f16 chunk cast loads, threshold, S clamp, output stores.
"""

import sys

sys.path.insert(0, "/opt/trn_rl_repo")

import numpy as np

N_FULL = 131072
Z = 128
M = 2048
P = 128
N_CORES = 8
LAM = 0.005
CHUNK = 16          # tiles per x pre-transpose chunk

_cache = {}


def _build(n_rows: int):
    import concourse.bass as bass
    import concourse.bacc as bacc
    import concourse.mybir as mybir
    import concourse.tile as tile

    f32 = mybir.dt.float32
    f16 = mybir.dt.float16
    Alu = mybir.AluOpType
    Act = mybir.ActivationFunctionType

    NT = n_rows // P
    assert n_rows % P == 0 and NT % CHUNK == 0
    NC_CH = M // P      # 16 mem chunks
    HB = M // 2         # 1024: exp half width

    nc = bacc.Bacc("TRN2", target_bir_lowering=False, debug=False, num_devices=1)
    x_d = nc.dram_tensor("x", [n_rows, Z], f32, kind="ExternalInput")
    mem_d = nc.dram_tensor("mem", [M, Z], f32, kind="ExternalInput")
    out_d = nc.dram_tensor("out", [n_rows, Z], f32, kind="ExternalOutput")

    with tile.TileContext(nc) as tc:
        with (
            tc.tile_pool(name="consts", bufs=1) as consts,
            tc.tile_pool(name="xcp", bufs=2) as xcp,
            tc.tile_pool(name="xtp", bufs=2) as xtp,
            tc.tile_pool(name="ep", bufs=6) as ep,
            tc.tile_pool(name="gp", bufs=6) as gp,
            tc.tile_pool(name="gtp", bufs=6) as gtp,
            tc.tile_pool(name="scal", bufs=16) as scal,
            tc.tile_pool(name="outp", bufs=6) as outp,
            tc.tile_pool(name="lps", bufs=3, space="PSUM") as lps,
            tc.tile_pool(name="ops", bufs=2, space="PSUM") as ops,
        ):
            # ---------- preamble ----------
            # mem as f16 chunks: mh[p, c, z] = mem[c*128+p, z]  (mm2 moving)
            mh = consts.tile([P, NC_CH, Z], f16)
            nc.gpsimd.dma_start(
                mh[:], mem_d.ap().rearrange("(c p) z -> p c z", p=P)
            )
            # mem^T via DMA XBAR: mhT[z, c, p] = mem[c*128+p, z] -> flat [z, m]
            mhT = consts.tile([P, NC_CH, P], f16)
            nc.sync.dma_start_transpose(mhT[:], mh[:])
            mhT_flat = mhT[:].rearrange("z c p -> z (c p)")

            # x: cast-load + XBAR-transpose ALL chunks up front.
            # xT[c][z, j, n] = x[c*2048 + j*128 + n, z]
            xT = {}
            for c in range(NT // CHUNK):
                r0 = c * CHUNK * P
                xc = xcp.tile([P, CHUNK, Z], f16, tag="xc", name="xc")
                nc.gpsimd.dma_start(
                    xc[:],
                    x_d.ap()[r0:r0 + CHUNK * P, :].rearrange(
                        "(j p) z -> p j z", p=P
                    ),
                )
                xT[c] = xtp.tile([P, CHUNK, P], f16, tag="xT", name="xT",
                                 bufs=NT // CHUNK)
                nc.sync.dma_start_transpose(xT[c][:], xc[:])

            # ---------- pipeline state ----------
            st = [dict() for _ in range(NT)]

            def stage_mm1(i):
                s = st[i]
                s["logits"] = []
                for h in range(2):
                    logits = lps.tile([P, HB], f32, tag="logits", name="logits")
                    for b in range(2):
                        ii = nc.tensor.matmul(
                            logits[:, b * 512:(b + 1) * 512],
                            xT[i // CHUNK][:, i % CHUNK, :],
                            mhT_flat[:, h * HB + b * 512: h * HB + (b + 1) * 512],
                            start=True, stop=True,
                        )
                        if h > 0 or b > 0:
                            ii.ins.ldweights = False
                    s["logits"].append(logits)

            def stage_exp(i):
                s = st[i]
                s["e"] = ep.tile([P, M], f16, tag="e", name="e")
                s["s1h"] = scal.tile([P, 2], f32, tag="s1h", name="s1h")
                nc.scalar.activation(
                    s["e"][:, 0:HB], s["logits"][0][:],
                    Act.Exp, scale=2.0,
                    accum_out=s["s1h"][:, 0:1],
                )
                nc.scalar.activation(
                    s["e"][:, HB:M], s["logits"][1][:],
                    Act.Exp, scale=2.0,
                    accum_out=s["s1h"][:, 1:2],
                )
                s.pop("logits")

            def stage_t(i):
                # t = lam * (s1h[0] + s1h[1])
                s = st[i]
                s1 = scal.tile([P, 1], f32, tag="s1", name="s1")
                nc.gpsimd.tensor_tensor(
                    out=s1[:], in0=s["s1h"][:, 0:1], in1=s["s1h"][:, 1:2],
                    op=Alu.add,
                )
                s["t"] = scal.tile([P, 1], f32, tag="t", name="t")
                nc.gpsimd.tensor_scalar_mul(s["t"][:], s1[:], LAM)
                s.pop("s1h")

            def stage_stt(i):
                s = st[i]
                s["g"] = gp.tile([P, M], f16, tag="g", name="g")
                S = scal.tile([P, 1], f32, tag="S", name="S")
                nc.vector.scalar_tensor_tensor(
                    out=s["g"][:], in0=s["e"][:], scalar=s["t"][:], in1=s["e"][:],
                    op0=Alu.is_gt, op1=Alu.mult, accum_out=S[:],
                )
                Sc = scal.tile([P, 1], f32, tag="Sc", name="Sc")
                nc.gpsimd.tensor_scalar_max(Sc[:], S[:], 1e-32)
                s["rS"] = scal.tile([P, 1], f32, tag="rS", name="rS")
                nc.vector.reciprocal(s["rS"][:], Sc[:])
                s.pop("e")
                s.pop("t")

            def stage_gt(i):
                # g [n, m] -> gT[p, c, n] = g[n, c*128+p] via DMA XBAR
                s = st[i]
                s["gT"] = gtp.tile([P, NC_CH, P], f16, tag="gT", name="gT")
                nc.sync.dma_start_transpose(s["gT"][:], s["g"][:])
                s.pop("g")

            def stage_mm2(i):
                # out[n, z] += gT_c^T @ mem_c  (gT chunk stationary)
                s = st[i]
                out_ps = ops.tile([P, 512], f32, tag="out_ps", name="out_ps")
                s["out_ps"] = out_ps
                for c in range(NC_CH):
                    nc.tensor.matmul(
                        out_ps[:, 0:Z], s["gT"][:, c, :], mh[:, c, :],
                        start=(c == 0), stop=(c == NC_CH - 1),
                    )
                s.pop("gT")

            def stage_fin(i):
                s = st[i]
                fin = outp.tile([P, Z], f32, tag="fin", name="fin")
                nc.vector.tensor_scalar_mul(fin[:], s["out_ps"][:, 0:Z], s["rS"][:])
                r0 = i * P
                nc.gpsimd.dma_start(out_d.ap()[r0:r0 + P, :], fin[:])
                s.pop("out_ps")
                s.pop("rS")

            # ---------- software-pipelined emission ----------
            SK_MM1, SK_EXP, SK_T, SK_STT, SK_GT = 2, 3, 4, 5, 6
            SK_MM2, SK_FIN = 8, 9
            LAST = SK_FIN

            stages = [
                (SK_MM2, stage_mm2),
                (SK_MM1, stage_mm1),
                (SK_EXP, stage_exp),
                (SK_T, stage_t),
                (SK_STT, stage_stt),
                (SK_GT, stage_gt),
                (SK_FIN, stage_fin),
            ]
            for s_idx in range(NT + LAST):
                for skew, fn in stages:
                    i = s_idx - skew
                    if 0 <= i < NT:
                        fn(i)

    nc.compile()
    return nc


def _get_nc(n_rows: int):
    if n_rows not in _cache:
        _cache[n_rows] = _build(n_rows)
    return _cache[n_rows]


def kernel(x: np.ndarray, mem: np.ndarray) -> np.ndarray:
    from concourse.bass_utils import run_bass_kernel_spmd

    x = np.ascontiguousarray(np.asarray(x, dtype=np.float32))
    mem = np.ascontiguousarray(np.asarray(mem, dtype=np.float32))
    n = x.shape[0]
    assert n % N_CORES == 0
    n_loc = n // N_CORES
    nc = _get_nc(n_loc)
    in_maps = [
        {"x": x[i * n_loc:(i + 1) * n_loc], "mem": mem} for i in range(N_CORES)
    ]
    # transient NRT/device errors happen occasionally; retry a couple times
    last_err = None
    for _ in range(3):
        try:
            res = run_bass_kernel_spmd(nc, in_maps, list(range(N_CORES)))
            break
        except Exception as err:  # noqa: BLE001
            last_err = err
            import time as _time
            _time.sleep(10)
    else:
        raise last_err
    out = np.concatenate([r["out"] for r in res.results], axis=0)
    return out.astype(np.float32)


# revision 13
# speedup vs baseline: 1.0048x; 1.0048x over previous
"""Trainium2 Bass kernel for nn_MemoryUnit (scatter_memory).

Computes: att = softmax(x @ mem.T / 0.5); att = hard_shrink_relu(att, 0.005);
att = att / max(L1(att), eps); out = att @ mem.

Sharding: data-parallel over N across 8 cores; mem replicated per core.

Per 128-row tile (m = 2048 memory slots), with the softmax denominator and
any per-row positive scale cancelled by the L1 renormalization:
  logits = x16 @ mem16.T            (single fp16 matmul, fp32 PSUM accum)
  e = exp(2*logits)                 (f16; ACT, rowsums via ACT accumulators)
  t = lam * rowsum(e)
  g = e * (e > t)                   (one DVE pass, accum S = rowsum(g))
  out = (g^T)^T @ mem / max(S, tiny)

Engine placement:
  - PE: mm1 + mm2 only (mm2 uses gT chunks as the stationary operand so the
    output is out[n, z] directly - no output transpose).
  - DMA XBAR (sync queue): x pre-transpose in 16-tile chunks, g transpose.
  - ACT: the two exp halves + their accumulator reads (rowsums).
  - DVE: the mask pass (scalar_tensor_tensor), reciprocal, final scale.
  - GPSIMD: f32->f16 chunk cast loads, threshold, S clamp, output stores.
"""

import sys

sys.path.insert(0, "/opt/trn_rl_repo")

import numpy as np

N_FULL = 131072
Z = 128
M = 2048
P = 128
N_CORES = 8
LAM = 0.005
CHUNK = 16          # tiles per x pre-transpose chunk

_cache = {}


def _build(n_rows: int):
    import concourse.bass as bass
    import concourse.bacc as bacc
    import concourse.mybir as mybir
    import concourse.tile as tile

    f32 = mybir.dt.float32
    f16 = mybir.dt.float16
    Alu = mybir.AluOpType
    Act = mybir.ActivationFunctionType

    NT = n_rows // P
    assert n_rows % P == 0 and NT % CHUNK == 0
    NC_CH = M // P      # 16 mem chunks
    HB = M // 2         # 1024: exp half width

    nc = bacc.Bacc("TRN2", target_bir_lowering=False, debug=False, num_devices=1)
    x_d = nc.dram_tensor("x", [n_rows, Z], f32, kind="ExternalInput")
    mem_d = nc.dram_tensor("mem", [M, Z], f32, kind="ExternalInput")
    out_d = nc.dram_tensor("out", [n_rows, Z], f32, kind="ExternalOutput")

    with tile.TileContext(nc) as tc:
        with (
            tc.tile_pool(name="consts", bufs=1) as consts,
            tc.tile_pool(name="xcp", bufs=2) as xcp,
            tc.tile_pool(name="xtp", bufs=2) as xtp,
            tc.tile_pool(name="ep", bufs=6) as ep,
            tc.tile_pool(name="gp", bufs=6) as gp,
            tc.tile_pool(name="gtp", bufs=6) as gtp,
            tc.tile_pool(name="scal", bufs=16) as scal,
            tc.tile_pool(name="outp", bufs=6) as outp,
            tc.tile_pool(name="lps", bufs=3, space="PSUM") as lps,
            tc.tile_pool(name="ops", bufs=2, space="PSUM") as ops,
        ):
            # ---------- preamble ----------
            # mem as f16 chunks: mh[p, c, z] = mem[c*128+p, z]  (mm2 moving)
            # plus the f16 residual ml = mem - mh for a second mm1 limb.
            mem_sb = consts.tile([P, NC_CH, Z], f32)
            nc.sync.dma_start(
                mem_sb[:], mem_d.ap().rearrange("(c p) z -> p c z", p=P)
            )
            mh = consts.tile([P, NC_CH, Z], f16)
            nc.vector.tensor_copy(out=mh[:], in_=mem_sb[:])
            ml = consts.tile([P, NC_CH, Z], f16)
            nc.vector.tensor_tensor(
                out=ml[:], in0=mem_sb[:], in1=mh[:], op=Alu.subtract
            )
            # mem^T via DMA XBAR: mhT[z, c, p] = mem[c*128+p, z] -> flat [z, m]
            mhT = consts.tile([P, NC_CH, P], f16)
            nc.sync.dma_start_transpose(mhT[:], mh[:])
            mhT_flat = mhT[:].rearrange("z c p -> z (c p)")
            mlT = consts.tile([P, NC_CH, P], f16)
            nc.sync.dma_start_transpose(mlT[:], ml[:])
            mlT_flat = mlT[:].rearrange("z c p -> z (c p)")

            # x: cast-load + XBAR-transpose ALL chunks up front.
            # xT[c][z, j, n] = x[c*2048 + j*128 + n, z]
            xT = {}
            for c in range(NT // CHUNK):
                r0 = c * CHUNK * P
                xc = xcp.tile([P, CHUNK, Z], f16, tag="xc", name="xc")
                nc.gpsimd.dma_start(
                    xc[:],
                    x_d.ap()[r0:r0 + CHUNK * P, :].rearrange(
                        "(j p) z -> p j z", p=P
                    ),
                )
                xT[c] = xtp.tile([P, CHUNK, P], f16, tag="xT", name="xT",
                                 bufs=NT // CHUNK)
                nc.sync.dma_start_transpose(xT[c][:], xc[:])

            # ---------- pipeline state ----------
            st = [dict() for _ in range(NT)]

            def stage_mm1(i):
                # logits = xh^T^T @ (mh + ml)^T: two limbs per 512-col bank,
                # same stationary (xT) for all 8 matmuls -> one weight load.
                s = st[i]
                xst = xT[i // CHUNK][:, i % CHUNK, :]
                s["logits"] = []
                first = True
                for h in range(2):
                    logits = lps.tile([P, HB], f32, tag="logits", name="logits")
                    for b in range(2):
                        sl = slice(h * HB + b * 512, h * HB + (b + 1) * 512)
                        for limb, mt in enumerate((mhT_flat, mlT_flat)):
                            ii = nc.tensor.matmul(
                                logits[:, b * 512:(b + 1) * 512],
                                xst, mt[:, sl],
                                start=(limb == 0), stop=(limb == 1),
                            )
                            if not first:
                                ii.ins.ldweights = False
                            first = False
                    s["logits"].append(logits)

            def stage_exp(i):
                s = st[i]
                s["e"] = ep.tile([P, M], f16, tag="e", name="e")
                s["s1h"] = scal.tile([P, 2], f32, tag="s1h", name="s1h")
                nc.scalar.activation(
                    s["e"][:, 0:HB], s["logits"][0][:],
                    Act.Exp, scale=2.0,
                    accum_out=s["s1h"][:, 0:1],
                )
                nc.scalar.activation(
                    s["e"][:, HB:M], s["logits"][1][:],
                    Act.Exp, scale=2.0,
                    accum_out=s["s1h"][:, 1:2],
                )
                s.pop("logits")

            def stage_t(i):
                # t = lam * (s1h[0] + s1h[1])
                s = st[i]
                s1 = scal.tile([P, 1], f32, tag="s1", name="s1")
                nc.gpsimd.tensor_tensor(
                    out=s1[:], in0=s["s1h"][:, 0:1], in1=s["s1h"][:, 1:2],
                    op=Alu.add,
                )
                s["t"] = scal.tile([P, 1], f32, tag="t", name="t")
                nc.gpsimd.tensor_scalar_mul(s["t"][:], s1[:], LAM)
                s.pop("s1h")

            def stage_stt(i):
                s = st[i]
                s["g"] = gp.tile([P, M], f16, tag="g", name="g")
                S = scal.tile([P, 1], f32, tag="S", name="S")
                nc.vector.scalar_tensor_tensor(
                    out=s["g"][:], in0=s["e"][:], scalar=s["t"][:], in1=s["e"][:],
                    op0=Alu.is_gt, op1=Alu.mult, accum_out=S[:],
                )
                Sc = scal.tile([P, 1], f32, tag="Sc", name="Sc")
                nc.gpsimd.tensor_scalar_max(Sc[:], S[:], 1e-32)
                s["rS"] = scal.tile([P, 1], f32, tag="rS", name="rS")
                nc.vector.reciprocal(s["rS"][:], Sc[:])
                s.pop("e")
                s.pop("t")

            def stage_gt(i):
                # g [n, m] -> gT[p, c, n] = g[n, c*128+p] via DMA XBAR
                s = st[i]
                s["gT"] = gtp.tile([P, NC_CH, P], f16, tag="gT", name="gT")
                nc.sync.dma_start_transpose(s["gT"][:], s["g"][:])
                s.pop("g")

            def stage_mm2(i):
                # out[n, z] += gT_c^T @ mem_c  (gT chunk stationary)
                s = st[i]
                out_ps = ops.tile([P, 512], f32, tag="out_ps", name="out_ps")
                s["out_ps"] = out_ps
                for c in range(NC_CH):
                    nc.tensor.matmul(
                        out_ps[:, 0:Z], s["gT"][:, c, :], mh[:, c, :],
                        start=(c == 0), stop=(c == NC_CH - 1),
                    )
                s.pop("gT")

            def stage_fin(i):
                s = st[i]
                fin = outp.tile([P, Z], f32, tag="fin", name="fin")
                nc.vector.tensor_scalar_mul(fin[:], s["out_ps"][:, 0:Z], s["rS"][:])
                r0 = i * P
                nc.gpsimd.dma_start(out_d.ap()[r0:r0 + P, :], fin[:])
                s.pop("out_ps")
                s.pop("rS")

            # ---------- software-pipelined emission ----------
            SK_MM1, SK_EXP, SK_T, SK_STT, SK_GT = 2, 3, 4, 5, 6
            SK_MM2, SK_FIN = 8, 9
            LAST = SK_FIN

            stages = [
                (SK_MM2, stage_mm2),
                (SK_MM1, stage_mm1),
                (SK_EXP, stage_exp),
                (SK_T, stage_t),
                (SK_STT, stage_stt),
                (SK_GT, stage_gt),
                (SK_FIN, stage_fin),
            ]
            for s_idx in range(NT + LAST):
                for skew, fn in stages:
                    i = s_idx - skew
                    if 0 <= i < NT:
                        fn(i)

    nc.compile()
    return nc


def _get_nc(n_rows: int):
    if n_rows not in _cache:
        _cache[n_rows] = _build(n_rows)
    return _cache[n_rows]


def kernel(x: np.ndarray, mem: np.ndarray) -> np.ndarray:
    from concourse.bass_utils import run_bass_kernel_spmd

    x = np.ascontiguousarray(np.asarray(x, dtype=np.float32))
    mem = np.ascontiguousarray(np.asarray(mem, dtype=np.float32))
    n = x.shape[0]
    assert n % N_CORES == 0
    n_loc = n // N_CORES
    nc = _get_nc(n_loc)
    in_maps = [
        {"x": x[i * n_loc:(i + 1) * n_loc], "mem": mem} for i in range(N_CORES)
    ]
    # transient NRT/device errors happen occasionally; retry a couple times
    last_err = None
    for _ in range(3):
        try:
            res = run_bass_kernel_spmd(nc, in_maps, list(range(N_CORES)))
            break
        except Exception as err:  # noqa: BLE001
            last_err = err
            import time as _time
            _time.sleep(10)
    else:
        raise last_err
    out = np.concatenate([r["out"] for r in res.results], axis=0)
    return out.astype(np.float32)


# revision 14
# speedup vs baseline: 1.0211x; 1.0162x over previous
"""Trainium2 Bass kernel for nn_MemoryUnit (scatter_memory).

Computes: att = softmax(x @ mem.T / 0.5); att = hard_shrink_relu(att, 0.005);
att = att / max(L1(att), eps); out = att @ mem.

Sharding: data-parallel over N across 8 cores; mem replicated per core.

Per 128-row tile (m = 2048 memory slots), with the softmax denominator and
any per-row positive scale cancelled by the L1 renormalization:
  logits = x16 @ mem16.T            (single fp16 matmul, fp32 PSUM accum)
  e = exp(2*logits)                 (f16; ACT, rowsums via ACT accumulators)
  t = lam * rowsum(e)
  g = e * (e > t)                   (one DVE pass, accum S = rowsum(g))
  out = (g^T)^T @ mem / max(S, tiny)

Engine placement:
  - PE: mm1 + mm2 only (mm2 uses gT chunks as the stationary operand so the
    output is out[n, z] directly - no output transpose).
  - DMA XBAR (sync queue): x pre-transpose in 16-tile chunks, g transpose.
  - ACT: the two exp halves + their accumulator reads (rowsums).
  - DVE: the mask pass (scalar_tensor_tensor), reciprocal, final scale.
  - GPSIMD: f32->f16 chunk cast loads, threshold, S clamp, output stores.
"""

import sys

sys.path.insert(0, "/opt/trn_rl_repo")

import numpy as np

N_FULL = 131072
Z = 128
M = 2048
P = 128
N_CORES = 8
LAM = 0.005
CHUNK = 16          # tiles per x pre-transpose chunk

_cache = {}


def _build(n_rows: int):
    import concourse.bass as bass
    import concourse.bacc as bacc
    import concourse.mybir as mybir
    import concourse.tile as tile

    f32 = mybir.dt.float32
    f16 = mybir.dt.float16
    Alu = mybir.AluOpType
    Act = mybir.ActivationFunctionType

    NT = n_rows // P
    assert n_rows % P == 0 and NT % CHUNK == 0
    NC_CH = M // P      # 16 mem chunks
    HB = M // 2         # 1024: exp half width

    nc = bacc.Bacc("TRN2", target_bir_lowering=False, debug=False, num_devices=1)
    x_d = nc.dram_tensor("x", [n_rows, Z], f32, kind="ExternalInput")
    mem_d = nc.dram_tensor("mem", [M, Z], f32, kind="ExternalInput")
    out_d = nc.dram_tensor("out", [n_rows, Z], f32, kind="ExternalOutput")

    with tile.TileContext(nc) as tc:
        with (
            tc.tile_pool(name="consts", bufs=1) as consts,
            tc.tile_pool(name="xcp", bufs=2) as xcp,
            tc.tile_pool(name="xtp", bufs=2) as xtp,
            tc.tile_pool(name="ep", bufs=6) as ep,
            tc.tile_pool(name="gp", bufs=6) as gp,
            tc.tile_pool(name="gtp", bufs=8) as gtp,
            tc.tile_pool(name="scal", bufs=16) as scal,
            tc.tile_pool(name="outp", bufs=6) as outp,
            tc.tile_pool(name="lps", bufs=3, space="PSUM") as lps,
            tc.tile_pool(name="ops", bufs=2, space="PSUM") as ops,
        ):
            # ---------- preamble ----------
            # mem as f16 chunks: mh[p, c, z] = mem[c*128+p, z]  (mm2 moving)
            # plus the f16 residual ml = mem - mh for a second mm1 limb.
            mem_sb = consts.tile([P, NC_CH, Z], f32)
            nc.sync.dma_start(
                mem_sb[:], mem_d.ap().rearrange("(c p) z -> p c z", p=P)
            )
            mh = consts.tile([P, NC_CH, Z], f16)
            nc.vector.tensor_copy(out=mh[:], in_=mem_sb[:])
            ml = consts.tile([P, NC_CH, Z], f16)
            nc.vector.tensor_tensor(
                out=ml[:], in0=mem_sb[:], in1=mh[:], op=Alu.subtract
            )
            # mem^T via DMA XBAR: mhT[z, c, p] = mem[c*128+p, z] -> flat [z, m]
            mhT = consts.tile([P, NC_CH, P], f16)
            nc.sync.dma_start_transpose(mhT[:], mh[:])
            mhT_flat = mhT[:].rearrange("z c p -> z (c p)")
            mlT = consts.tile([P, NC_CH, P], f16)
            nc.sync.dma_start_transpose(mlT[:], ml[:])
            mlT_flat = mlT[:].rearrange("z c p -> z (c p)")

            # x: cast-load + XBAR-transpose ALL chunks up front.
            # xT[c][z, j, n] = x[c*2048 + j*128 + n, z]
            xT = {}
            for c in range(NT // CHUNK):
                r0 = c * CHUNK * P
                xc = xcp.tile([P, CHUNK, Z], f16, tag="xc", name="xc")
                nc.gpsimd.dma_start(
                    xc[:],
                    x_d.ap()[r0:r0 + CHUNK * P, :].rearrange(
                        "(j p) z -> p j z", p=P
                    ),
                )
                xT[c] = xtp.tile([P, CHUNK, P], f16, tag="xT", name="xT",
                                 bufs=NT // CHUNK)
                nc.sync.dma_start_transpose(xT[c][:], xc[:])

            # ---------- pipeline state ----------
            st = [dict() for _ in range(NT)]

            def stage_mm1(i):
                # logits = xh^T^T @ (mh + ml)^T: two limbs per 512-col bank,
                # same stationary (xT) for all 8 matmuls -> one weight load.
                s = st[i]
                xst = xT[i // CHUNK][:, i % CHUNK, :]
                s["logits"] = []
                first = True
                for h in range(2):
                    logits = lps.tile([P, HB], f32, tag="logits", name="logits")
                    for b in range(2):
                        sl = slice(h * HB + b * 512, h * HB + (b + 1) * 512)
                        for limb, mt in enumerate((mhT_flat, mlT_flat)):
                            ii = nc.tensor.matmul(
                                logits[:, b * 512:(b + 1) * 512],
                                xst, mt[:, sl],
                                start=(limb == 0), stop=(limb == 1),
                            )
                            if not first:
                                ii.ins.ldweights = False
                            first = False
                    s["logits"].append(logits)

            def stage_exp(i):
                s = st[i]
                s["e"] = ep.tile([P, M], f16, tag="e", name="e")
                s["s1h"] = scal.tile([P, 2], f32, tag="s1h", name="s1h")
                nc.scalar.activation(
                    s["e"][:, 0:HB], s["logits"][0][:],
                    Act.Exp, scale=2.0,
                    accum_out=s["s1h"][:, 0:1],
                )
                nc.scalar.activation(
                    s["e"][:, HB:M], s["logits"][1][:],
                    Act.Exp, scale=2.0,
                    accum_out=s["s1h"][:, 1:2],
                )
                s.pop("logits")

            def stage_t(i):
                # t = lam * (s1h[0] + s1h[1])
                s = st[i]
                s1 = scal.tile([P, 1], f32, tag="s1", name="s1")
                nc.gpsimd.tensor_tensor(
                    out=s1[:], in0=s["s1h"][:, 0:1], in1=s["s1h"][:, 1:2],
                    op=Alu.add,
                )
                s["t"] = scal.tile([P, 1], f32, tag="t", name="t")
                nc.gpsimd.tensor_scalar_mul(s["t"][:], s1[:], LAM)
                s.pop("s1h")

            def stage_stt(i):
                s = st[i]
                s["g"] = gp.tile([P, M], f16, tag="g", name="g")
                S = scal.tile([P, 1], f32, tag="S", name="S")
                nc.vector.scalar_tensor_tensor(
                    out=s["g"][:], in0=s["e"][:], scalar=s["t"][:], in1=s["e"][:],
                    op0=Alu.is_gt, op1=Alu.mult, accum_out=S[:],
                )
                Sc = scal.tile([P, 1], f32, tag="Sc", name="Sc")
                nc.gpsimd.tensor_scalar_max(Sc[:], S[:], 1e-32)
                s["rS"] = scal.tile([P, 1], f32, tag="rS", name="rS")
                nc.vector.reciprocal(s["rS"][:], Sc[:])
                s.pop("e")
                s.pop("t")

            def stage_gt(i):
                # g [n, m] -> gT[p, c, n] = g[n, c*128+p] via DMA XBAR
                s = st[i]
                s["gT"] = gtp.tile([P, NC_CH, P], f16, tag="gT", name="gT")
                nc.sync.dma_start_transpose(s["gT"][:], s["g"][:])
                s.pop("g")

            def stage_mm2(i):
                # out[n, z] += gT_c^T @ mem_c  (gT chunk stationary)
                s = st[i]
                out_ps = ops.tile([P, 512], f32, tag="out_ps", name="out_ps")
                s["out_ps"] = out_ps
                for c in range(NC_CH):
                    nc.tensor.matmul(
                        out_ps[:, 0:Z], s["gT"][:, c, :], mh[:, c, :],
                        start=(c == 0), stop=(c == NC_CH - 1),
                    )
                s.pop("gT")

            def stage_fin(i):
                s = st[i]
                fin = outp.tile([P, Z], f32, tag="fin", name="fin")
                nc.vector.tensor_scalar_mul(fin[:], s["out_ps"][:, 0:Z], s["rS"][:])
                r0 = i * P
                nc.gpsimd.dma_start(out_d.ap()[r0:r0 + P, :], fin[:])
                s.pop("out_ps")
                s.pop("rS")

            # ---------- software-pipelined emission ----------
            SK_MM1, SK_EXP, SK_T, SK_STT, SK_GT = 2, 3, 4, 5, 6
            SK_MM2, SK_FIN = 12, 13
            LAST = SK_FIN

            stages = [
                (SK_MM1, stage_mm1),
                (SK_MM2, stage_mm2),
                (SK_EXP, stage_exp),
                (SK_T, stage_t),
                (SK_STT, stage_stt),
                (SK_GT, stage_gt),
                (SK_FIN, stage_fin),
            ]
            for s_idx in range(NT + LAST):
                for skew, fn in stages:
                    i = s_idx - skew
                    if 0 <= i < NT:
                        fn(i)

    nc.compile()
    return nc


def _get_nc(n_rows: int):
    if n_rows not in _cache:
        _cache[n_rows] = _build(n_rows)
    return _cache[n_rows]


def kernel(x: np.ndarray, mem: np.ndarray) -> np.ndarray:
    from concourse.bass_utils import run_bass_kernel_spmd

    x = np.ascontiguousarray(np.asarray(x, dtype=np.float32))
    mem = np.ascontiguousarray(np.asarray(mem, dtype=np.float32))
    n = x.shape[0]
    assert n % N_CORES == 0
    n_loc = n // N_CORES
    nc = _get_nc(n_loc)
    in_maps = [
        {"x": x[i * n_loc:(i + 1) * n_loc], "mem": mem} for i in range(N_CORES)
    ]
    # transient NRT/device errors happen occasionally; retry a couple times
    last_err = None
    for _ in range(3):
        try:
            res = run_bass_kernel_spmd(nc, in_maps, list(range(N_CORES)))
            break
        except Exception as err:  # noqa: BLE001
            last_err = err
            import time as _time
            _time.sleep(10)
    else:
        raise last_err
    out = np.concatenate([r["out"] for r in res.results], axis=0)
    return out.astype(np.float32)
